# revision 1
# baseline (speedup 1.0000x reference)
"""GumbelSelector Trainium2 kernel.

Math: h = relu(s @ W1 + b1); lo = h @ W2 + b2  (2 classes)
  dec  = (argmax(lo) == 1)  ==  (z > 0)         where z = h @ (W2[:,1]-W2[:,0]) + (b2[1]-b2[0])
  prob = softmax(lo)[..., 1] ==  sigmoid(z)
  Per-row correction (LB=1): if a row of dec is all zero, activate argmax(rnoise).

Sharding: data-parallel over batch B=64 -> 8 cores x 8 rows. Weights replicated.
Host pre-transposes each core's s shard to [D=256, 32768] so the DMA loads are
fully coalesced and the contraction dim lands on SBUF partitions directly.
"""

import sys

if "/opt/trn_rl_repo" not in sys.path:
    sys.path.insert(0, "/opt/trn_rl_repo")

import numpy as np

import concourse.bass as bass
import concourse.mybir as mybir
import concourse.tile as tile
from concourse import bacc
from concourse.bass_utils import run_bass_kernel_spmd

B, N, D = 64, 4096, 256
HID = D // 2  # 128
NCORES = 8
BPC = B // NCORES          # batch rows per core
TOK = BPC * N              # 32768 tokens per core
SLAB = 2048                # tokens per DMA slab (1 MiB per 128-partition load)
TS = 1024                  # tokens per compute tile (2 PSUM banks)
F32 = mybir.dt.float32

_NC = None


def _build_nc():
    nc = bacc.Bacc("TRN2", target_bir_lowering=False, debug=False)
    sT = nc.dram_tensor("sT", [D, TOK], F32, kind="ExternalInput")
    rn = nc.dram_tensor("rn", [BPC, N], F32, kind="ExternalInput")
    w1 = nc.dram_tensor("w1", [D, HID], F32, kind="ExternalInput")
    b1 = nc.dram_tensor("b1", [HID, 1], F32, kind="ExternalInput")
    w2d = nc.dram_tensor("w2d", [HID, 1], F32, kind="ExternalInput")
    b2d = nc.dram_tensor("b2d", [1, 1], F32, kind="ExternalInput")
    nb2d = nc.dram_tensor("nb2d", [1, 1], F32, kind="ExternalInput")
    dec = nc.dram_tensor("dec", [1, TOK], F32, kind="ExternalOutput")
    prob = nc.dram_tensor("prob", [1, TOK], F32, kind="ExternalOutput")

    AF = mybir.ActivationFunctionType
    ALU = mybir.AluOpType

    with tile.TileContext(nc) as tc:
        with (
            tc.tile_pool(name="consts", bufs=1) as consts,
            tc.tile_pool(name="io8", bufs=1) as io8,
            tc.tile_pool(name="sload", bufs=3) as sload,
            tc.tile_pool(name="hpool", bufs=3) as hpool,
            tc.tile_pool(name="cpool", bufs=4) as cpool,
            tc.tile_pool(name="phpool", bufs=2, space=bass.MemorySpace.PSUM) as phpool,
            tc.tile_pool(name="pzpool", bufs=2, space=bass.MemorySpace.PSUM) as pzpool,
        ):
            w1a = consts.tile([128, HID], F32)
            nc.sync.dma_start(w1a[:], w1[0:128, :])
            w1b = consts.tile([128, HID], F32)
            nc.sync.dma_start(w1b[:], w1[128:256, :])
            b1s = consts.tile([HID, 1], F32)
            nc.sync.dma_start(b1s[:], b1[:])
            w2s = consts.tile([HID, 1], F32)
            nc.sync.dma_start(w2s[:], w2d[:])
            b2s = consts.tile([1, 1], F32)
            nc.sync.dma_start(b2s[:], b2d[:])
            nb2s = consts.tile([1, 1], F32)
            nc.sync.dma_start(nb2s[:], nb2d[:])
            rns = io8.tile([BPC, N], F32)
            nc.sync.dma_start(rns[:], rn[:])

            # engines may only address base partition 0/32/64/96, so compute
            # dec/prob chunks on partition 0; prob streams straight to DRAM,
            # dec chunks are DMA'd into row-layout for the row fixup
            dec8 = io8.tile([BPC, N], F32)

            for si in range(TOK // SLAB):
                off = si * SLAB
                sa = sload.tile([128, SLAB], F32, tag="sa")
                sb = sload.tile([128, SLAB], F32, tag="sb")
                nc.sync.dma_start(sa[:], sT[0:128, off : off + SLAB])
                nc.sync.dma_start(sb[:], sT[128:256, off : off + SLAB])
                for half in range(SLAB // TS):
                    toff = off + half * TS
                    hoff = half * TS
                    ph = phpool.tile([128, TS], F32)
                    # same stationary back to back to minimize LDWEIGHTS swaps
                    nc.tensor.matmul(ph[:, 0:512], w1a[:], sa[:, hoff : hoff + 512],
                                     start=True, stop=False)
                    nc.tensor.matmul(ph[:, 512:1024], w1a[:], sa[:, hoff + 512 : hoff + 1024],
                                     start=True, stop=False)
                    nc.tensor.matmul(ph[:, 0:512], w1b[:], sb[:, hoff : hoff + 512],
                                     start=False, stop=True)
                    nc.tensor.matmul(ph[:, 512:1024], w1b[:], sb[:, hoff + 512 : hoff + 1024],
                                     start=False, stop=True)
                    h = hpool.tile([128, TS], F32)
                    nc.scalar.activation(h[:], ph[:], AF.Relu, bias=b1s[:])
                    pz = pzpool.tile([1, TS], F32)
                    nc.tensor.matmul(pz[0:1, 0:512], w2s[:], h[:, 0:512],
                                     start=True, stop=True)
                    nc.tensor.matmul(pz[0:1, 512:1024], w2s[:], h[:, 512:1024],
                                     start=True, stop=True)
                    pc = cpool.tile([1, TS], F32, tag="pc")
                    nc.scalar.activation(pc[:], pz[0:1, :], AF.Sigmoid, bias=b2s[:])
                    nc.sync.dma_start(prob[0:1, toff : toff + TS], pc[:])
                    dc = cpool.tile([1, TS], F32, tag="dc")
                    nc.vector.tensor_scalar(dc[:], pz[0:1, :], nb2s[:], None, ALU.is_gt)
                    b_row, col = toff // N, toff % N
                    nc.sync.dma_start(dec8[b_row : b_row + 1, col : col + TS], dc[:])

            # Row correction: rows with no active slot get argmax(rnoise) forced on.
            rmaxd = io8.tile([BPC, 1], F32)
            nc.vector.tensor_reduce(rmaxd[:], dec8[:], mybir.AxisListType.X, ALU.max)
            need = io8.tile([BPC, 1], F32)
            nc.vector.tensor_scalar(need[:], rmaxd[:], 0.0, None, ALU.is_equal)
            rmaxr = io8.tile([BPC, 1], F32)
            nc.vector.tensor_reduce(rmaxr[:], rns[:], mybir.AxisListType.X, ALU.max)
            fix = io8.tile([BPC, N], F32)
            nc.vector.tensor_scalar(fix[:], rns[:], rmaxr[:], need[:],
                                    ALU.is_equal, ALU.mult)
            decf = io8.tile([BPC, N], F32)
            nc.vector.tensor_max(decf[:], dec8[:], fix[:])

            for b in range(BPC):
                nc.sync.dma_start(dec[0:1, b * N : (b + 1) * N], decf[b : b + 1, :])

    nc.compile()
    return nc


def _get_nc():
    global _NC
    if _NC is None:
        _NC = _build_nc()
    return _NC


def _make_in_maps(s, W1, b1, W2, b2, rnoise):
    s = np.ascontiguousarray(s, dtype=np.float32)
    w1 = np.ascontiguousarray(W1, dtype=np.float32)
    b1c = np.ascontiguousarray(b1, dtype=np.float32).reshape(HID, 1)
    w2dc = np.ascontiguousarray(W2[:, 1] - W2[:, 0], dtype=np.float32).reshape(HID, 1)
    b2dv = np.float32(b2[1] - b2[0])
    b2dc = np.array([[b2dv]], dtype=np.float32)
    nb2dc = np.array([[-b2dv]], dtype=np.float32)
    rn = np.ascontiguousarray(rnoise, dtype=np.float32)

    # [NCORES, D, TOK] with the contraction dim outer -> coalesced loads
    sT = np.ascontiguousarray(
        s.reshape(NCORES, TOK, D).transpose(0, 2, 1)
    )
    return [
        {
            "sT": sT[c],
            "rn": rn.reshape(NCORES, BPC, N)[c],
            "w1": w1,
            "b1": b1c,
            "w2d": w2dc,
            "b2d": b2dc,
            "nb2d": nb2dc,
        }
        for c in range(NCORES)
    ]


def run(s, W1, b1, W2, b2, rnoise, trace=False):
    nc = _get_nc()
    in_maps = _make_in_maps(s, W1, b1, W2, b2, rnoise)
    res = run_bass_kernel_spmd(nc, in_maps, list(range(NCORES)), trace=trace)
    dec = np.concatenate(
        [r["dec"].reshape(BPC, N) for r in res.results], axis=0
    )
    prob = np.concatenate(
        [r["prob"].reshape(BPC, N) for r in res.results], axis=0
    )
    return (dec, prob), res


def kernel(s, W1, b1, W2, b2, rnoise):
    (dec, prob), _ = run(s, W1, b1, W2, b2, rnoise)
    return dec, prob



# revision 7
# speedup vs baseline: 1.2233x; 1.2233x over previous
"""GumbelSelector Trainium2 kernel.

Math: h = relu(s @ W1 + b1); lo = h @ W2 + b2  (2 classes)
  dec  = (argmax(lo) == 1)  ==  (z > 0)         where z = h @ (W2[:,1]-W2[:,0]) + (b2[1]-b2[0])
  prob = softmax(lo)[..., 1] ==  sigmoid(z)
  Per-row correction (LB=1): if a row of dec is all zero, activate argmax(rnoise).

Sharding: data-parallel over batch B=64 -> 8 cores x 8 rows. Weights replicated.
Host pre-transposes each core's s shard to [D=256, 32768] so the DMA loads are
fully coalesced and the contraction dim lands on SBUF partitions directly.
"""

import sys

if "/opt/trn_rl_repo" not in sys.path:
    sys.path.insert(0, "/opt/trn_rl_repo")

import numpy as np

import concourse.bass as bass
import concourse.mybir as mybir
import concourse.tile as tile
from concourse import bacc
from concourse.bass_utils import run_bass_kernel_spmd

B, N, D = 64, 4096, 256
HID = D // 2  # 128
NCORES = 8
BPC = B // NCORES          # batch rows per core
TOK = BPC * N              # 32768 tokens per core
SLAB = 2048                # tokens per DMA slab (1 MiB per 128-partition load)
TS = 1024                  # tokens per compute tile (2 PSUM banks)
F32 = mybir.dt.float32
F32R = mybir.dt.float32r   # fp32 data, 1 cycle/row on the PE at free-dim >= 256

_NC = None


def _build_nc():
    nc = bacc.Bacc("TRN2", target_bir_lowering=False, debug=False)
    # matmul operands are float32r (fp32 rounded to 11 explicit mantissa
    # bits): 1 PE cycle/row instead of 4 for plain fp32. Host pre-rounds.
    sT = nc.dram_tensor("sT", [D, TOK], F32R, kind="ExternalInput")
    rn = nc.dram_tensor("rn", [BPC, N], F32, kind="ExternalInput")
    w1 = nc.dram_tensor("w1", [D, HID], F32R, kind="ExternalInput")
    b1 = nc.dram_tensor("b1", [HID, 1], F32, kind="ExternalInput")
    w2d = nc.dram_tensor("w2d", [HID, 1], F32R, kind="ExternalInput")
    b2d = nc.dram_tensor("b2d", [1, 1], F32, kind="ExternalInput")
    nb2d = nc.dram_tensor("nb2d", [1, 1], F32, kind="ExternalInput")
    dec = nc.dram_tensor("dec", [1, TOK], F32, kind="ExternalOutput")
    prob = nc.dram_tensor("prob", [1, TOK], F32, kind="ExternalOutput")

    AF = mybir.ActivationFunctionType
    ALU = mybir.AluOpType

    with tile.TileContext(nc) as tc:
        with (
            tc.tile_pool(name="consts", bufs=1) as consts,
            tc.tile_pool(name="io8", bufs=1) as io8,
            tc.tile_pool(name="sload", bufs=3) as sload,
            tc.tile_pool(name="hpool", bufs=3) as hpool,
            tc.tile_pool(name="cpool", bufs=4) as cpool,
            tc.tile_pool(name="phpool", bufs=2, space=bass.MemorySpace.PSUM) as phpool,
            tc.tile_pool(name="pzpool", bufs=2, space=bass.MemorySpace.PSUM) as pzpool,
        ):
            w1a = consts.tile([128, HID], F32R)
            nc.sync.dma_start(w1a[:], w1[0:128, :])
            w1b = consts.tile([128, HID], F32R)
            nc.sync.dma_start(w1b[:], w1[128:256, :])
            b1s = consts.tile([HID, 1], F32)
            nc.sync.dma_start(b1s[:], b1[:])
            w2s = consts.tile([HID, 1], F32R)
            nc.sync.dma_start(w2s[:], w2d[:])
            b2s = consts.tile([1, 1], F32)
            nc.sync.dma_start(b2s[:], b2d[:])
            nb2s = consts.tile([1, 1], F32)
            nc.sync.dma_start(nb2s[:], nb2d[:])
            rns = io8.tile([BPC, N], F32)
            nc.sync.dma_start(rns[:], rn[:])

            # engines may only address base partition 0/32/64/96, so compute
            # dec/prob chunks on partition 0; prob streams straight to DRAM,
            # dec chunks are DMA'd into row-layout for the row fixup
            dec8 = io8.tile([BPC, N], F32)

            # rnoise argmax indicator is dec-independent: hoist it so it
            # overlaps the first slab loads instead of sitting in the tail
            rmaxr = io8.tile([BPC, 1], F32)
            nc.vector.tensor_reduce(rmaxr[:], rns[:], mybir.AxisListType.X, ALU.max)
            fixcand = io8.tile([BPC, N], F32)
            nc.vector.tensor_scalar(fixcand[:], rns[:], rmaxr[:], None, ALU.is_equal)

            for si in range(TOK // SLAB):
                off = si * SLAB
                sa = sload.tile([128, SLAB], F32R, tag="sa")
                sb = sload.tile([128, SLAB], F32R, tag="sb")
                nc.sync.dma_start(sa[:], sT[0:128, off : off + SLAB])
                nc.sync.dma_start(sb[:], sT[128:256, off : off + SLAB])
                for half in range(SLAB // TS):
                    toff = off + half * TS
                    hoff = half * TS
                    ph = phpool.tile([128, TS], F32)
                    # same stationary back to back to minimize LDWEIGHTS swaps
                    nc.tensor.matmul(ph[:, 0:512], w1a[:],
                                     sa[:, hoff : hoff + 512],
                                     start=True, stop=False)
                    nc.tensor.matmul(ph[:, 512:1024], w1a[:],
                                     sa[:, hoff + 512 : hoff + 1024],
                                     start=True, stop=False)
                    nc.tensor.matmul(ph[:, 0:512], w1b[:],
                                     sb[:, hoff : hoff + 512],
                                     start=False, stop=True)
                    nc.tensor.matmul(ph[:, 512:1024], w1b[:],
                                     sb[:, hoff + 512 : hoff + 1024],
                                     start=False, stop=True)
                    h = hpool.tile([128, TS], F32R)
                    nc.scalar.activation(h[:], ph[:], AF.Relu, bias=b1s[:])
                    pz = pzpool.tile([1, TS], F32)
                    nc.tensor.matmul(pz[0:1, 0:512], w2s[:],
                                     h[:, 0:512],
                                     start=True, stop=True)
                    nc.tensor.matmul(pz[0:1, 512:1024], w2s[:],
                                     h[:, 512:1024],
                                     start=True, stop=True)
                    pc = cpool.tile([1, TS], F32, tag="pc")
                    nc.scalar.activation(pc[:], pz[0:1, :], AF.Sigmoid, bias=b2s[:])
                    nc.sync.dma_start(prob[0:1, toff : toff + TS], pc[:])
                    dc = cpool.tile([1, TS], F32, tag="dc")
                    nc.vector.tensor_scalar(dc[:], pz[0:1, :], nb2s[:], None, ALU.is_gt)
                    b_row, col = toff // N, toff % N
                    nc.sync.dma_start(dec8[b_row : b_row + 1, col : col + TS], dc[:])

            # Row correction: rows with no active slot get argmax(rnoise) forced on.
            rmaxd = io8.tile([BPC, 1], F32)
            nc.vector.tensor_reduce(rmaxd[:], dec8[:], mybir.AxisListType.X, ALU.max)
            need = io8.tile([BPC, 1], F32)
            nc.vector.tensor_scalar(need[:], rmaxd[:], 0.0, None, ALU.is_equal)
            fix = io8.tile([BPC, N], F32)
            nc.vector.tensor_scalar(fix[:], fixcand[:], need[:], None, ALU.mult)
            decf = io8.tile([BPC, N], F32)
            nc.vector.tensor_max(decf[:], dec8[:], fix[:])

            for b in range(BPC):
                nc.sync.dma_start(dec[0:1, b * N : (b + 1) * N], decf[b : b + 1, :])

    nc.compile()
    return nc


def _get_nc():
    global _NC
    if _NC is None:
        _NC = _build_nc()
    return _NC


def _round_fp32r(x):
    # round-to-nearest-even at mantissa bit 12 (matches HW fp32_to_fp32r)
    b = np.ascontiguousarray(x, dtype=np.float32).view(np.uint32)
    r = (b + np.uint32(0x7FF) + ((b >> np.uint32(12)) & np.uint32(1))) & np.uint32(
        0xFFFFF000
    )
    return r.view(np.float32)


def _make_in_maps(s, W1, b1, W2, b2, rnoise):
    s = _round_fp32r(s)
    w1 = _round_fp32r(W1)
    b1c = np.ascontiguousarray(b1, dtype=np.float32).reshape(HID, 1)
    w2dc = _round_fp32r(
        np.asarray(W2[:, 1] - W2[:, 0], dtype=np.float32)
    ).reshape(HID, 1)
    b2dv = np.float32(b2[1] - b2[0])
    b2dc = np.array([[b2dv]], dtype=np.float32)
    nb2dc = np.array([[-b2dv]], dtype=np.float32)
    rn = np.ascontiguousarray(rnoise, dtype=np.float32)

    # [NCORES, D, TOK] with the contraction dim outer -> coalesced loads
    sT = np.ascontiguousarray(
        s.reshape(NCORES, TOK, D).transpose(0, 2, 1)
    )
    return [
        {
            "sT": sT[c],
            "rn": rn.reshape(NCORES, BPC, N)[c],
            "w1": w1,
            "b1": b1c,
            "w2d": w2dc,
            "b2d": b2dc,
            "nb2d": nb2dc,
        }
        for c in range(NCORES)
    ]


def run(s, W1, b1, W2, b2, rnoise, trace=False):
    nc = _get_nc()
    in_maps = _make_in_maps(s, W1, b1, W2, b2, rnoise)
    res = run_bass_kernel_spmd(nc, in_maps, list(range(NCORES)), trace=trace)
    dec = np.concatenate(
        [r["dec"].reshape(BPC, N) for r in res.results], axis=0
    )
    prob = np.concatenate(
        [r["prob"].reshape(BPC, N) for r in res.results], axis=0
    )
    return (dec, prob), res


def kernel(s, W1, b1, W2, b2, rnoise):
    (dec, prob), _ = run(s, W1, b1, W2, b2, rnoise)
    return dec, prob

